# revision 13
# baseline (speedup 1.0000x reference)
"""Trainium2 Bass kernel for causal multi-head attention.

Problem: B=2, S=2048, D=1024, H=16 heads (hd=64), fp32 in/out.
  qkv = x @ Wqkv + bqkv ; per-head causal softmax attention ; out = ctx @ Wo + bo

Sharding (8 NeuronCores): tensor-parallel over heads — 2 heads per core.
Each core computes q/k/v projections for its 2 heads (both batches), causal
attention, and its ctx^T slice [128 feat, B*S]. Four AllToAll exchanges
(one per 512-row output chunk) route each core's 128-feature slice of the
other cores' output rows; each core then computes the output projection for
its 512 rows (4 chunks of 128) with the full Wo. Host reassembles.

Numerics: bf16 matmul operands, fp32 PSUM accumulation. Softmax uses
exp without max-subtraction (scores are ~N(0,1) after the folded 1/sqrt(hd)
scale). The softmax denominator comes for free as a ones-column appended to
v in the attn@v matmul. All bias adds are K=1 PE matmuls into the PSUM
accumulation groups (biases are zero in this problem but kept for fidelity).
"""

import numpy as np
import ml_dtypes

B, S, D, H, NC = 2, 2048, 1024, 16, 8
HD = D // H            # 64
HPC = H // NC          # 2 heads per core
BS = B * S             # 4096
RPB = S // NC          # 256 output rows per core per batch
KC = D // 128          # 8 contraction chunks
SC = BS // 512         # 8 s-chunks for qkv projection
NQT = S // 512         # 4 q-tiles (512) per batch
NKT = S // 128         # 16 k-tiles (128) per batch
NCH = 2 * B            # 4 output chunks (b, half)

BF16 = ml_dtypes.bfloat16

_CACHE = {}


def _build_program():
    import concourse.bass as bass
    import concourse.mybir as mybir
    from concourse import bacc
    from concourse.tile import TileContext

    dt = mybir.dt
    f32, bf16 = dt.float32, dt.bfloat16
    ALU = mybir.AluOpType
    ACTF = mybir.ActivationFunctionType

    nc = bacc.Bacc("TRN2", target_bir_lowering=False, debug=False, num_devices=NC)

    xT = nc.dram_tensor("xT", [D, BS], bf16, kind="ExternalInput")
    wqk = nc.dram_tensor("wqk", [D, 256], bf16, kind="ExternalInput")
    wv = nc.dram_tensor("wv", [D, 128], bf16, kind="ExternalInput")
    wo = nc.dram_tensor("wo", [D, D], bf16, kind="ExternalInput")
    bqk = nc.dram_tensor("bqk", [1, 256], bf16, kind="ExternalInput")
    bv = nc.dram_tensor("bv", [1, 128], bf16, kind="ExternalInput")
    bo = nc.dram_tensor("bo", [1, D], bf16, kind="ExternalInput")
    mask = nc.dram_tensor("mask", [128, 896], bf16, kind="ExternalInput")
    out = nc.dram_tensor("out", [NCH * 128, D], f32, kind="ExternalOutput")

    # AllToAll buffers: for chunk g, block d of a2a_in holds my 128 features
    # for destination core d's 128 output rows; a2a_out block s holds core
    # s's 128 features for MY 128 rows of chunk g.
    a2a_in = [nc.dram_tensor(f"a2ain{g}", [NC * 128, 128], bf16)
              for g in range(NCH)]
    a2a_out = [nc.dram_tensor(f"a2aout{g}", [NC * 128, 128], bf16)
               for g in range(NCH)]

    with TileContext(nc) as tc:
        with (
            tc.tile_pool(name="const", bufs=1) as cpool,
            tc.tile_pool(name="big", bufs=1) as bigpool,
            tc.tile_pool(name="xstream", bufs=2) as xpool,
            tc.tile_pool(name="exp", bufs=1) as epool,
            tc.tile_pool(name="small", bufs=3) as spool,
            tc.tile_pool(name="agbuf", bufs=2) as agpool,
            tc.tile_pool(name="psA", bufs=2, space="PSUM") as psA,   # 2x [128,1536]
            tc.tile_pool(name="psB", bufs=2, space="PSUM") as psB,   # 2x [128,512]
        ):
            # ---- constants / weights to SBUF ----
            wqk_sb = cpool.tile([128, KC, 256], bf16, tag="wqk")
            nc.sync.dma_start(wqk_sb[:], wqk.rearrange("(ko p) m -> p ko m", p=128))
            wv_sb = cpool.tile([128, KC, 128], bf16, tag="wv")
            nc.sync.dma_start(wv_sb[:], wv.rearrange("(ko p) m -> p ko m", p=128))
            wo_sb = cpool.tile([128, KC, D], bf16, tag="wo")
            nc.sync.dma_start(wo_sb[:], wo.rearrange("(ko p) m -> p ko m", p=128))
            bqk_sb = cpool.tile([1, 256], bf16, tag="bqk")
            nc.sync.dma_start(bqk_sb[:], bqk[:])
            bv_sb = cpool.tile([1, 128], bf16, tag="bv")
            nc.sync.dma_start(bv_sb[:], bv[:])
            bo_sb = cpool.tile([1, D], bf16, tag="bo")
            nc.sync.dma_start(bo_sb[:], bo[:])
            mask_sb = cpool.tile([128, 896], bf16, tag="mask")
            nc.sync.dma_start(mask_sb[:], mask[:])
            ones_sb = cpool.tile([1, 512], bf16, tag="ones")
            nc.vector.memset(ones_sb[:], 1.0)
            zrow_sb = cpool.tile([1, 65], bf16, tag="zrow")
            nc.vector.memset(zrow_sb[:], 0.0)

            # ---- persistent activations ----
            qT_sb = bigpool.tile([128, BS], bf16, tag="qT")   # [2*64 feat, B*S]
            kT_sb = bigpool.tile([128, BS], bf16, tag="kT")
            # v natural layout + ones cols: per 128-row chunk:
            #   [v_h0(0:64) | ones(64) | v_h1(65:129) | ones(129)]
            v_sb = bigpool.tile([128, BS // 128, 130], bf16, tag="v")
            ctxT_sb = bigpool.tile([128, BS], bf16, tag="ctxT")

            nc.vector.memset(v_sb[:, :, 64:65], 1.0)
            nc.vector.memset(v_sb[:, :, 129:130], 1.0)

            # ---- phase 1: qkv projections ----
            scope1 = nc.named_scope("qkv"); scope1.__enter__()
            xT_r = xT.rearrange("(ko p) s -> p ko s", p=128)

            def emit_qkv_chunk(sc):
                xt = xpool.tile([128, KC, 512], bf16, tag="xt")
                nc.sync.dma_start(xt[:], xT_r[:, :, sc * 512:(sc + 1) * 512])

                ps_q = psA.tile([128, 1536], f32, tag="psA", name="ps_q")[:, :512]
                ps_k = psA.tile([128, 1536], f32, tag="psA", name="ps_k")[:, :512]
                for kk in range(KC):
                    nc.tensor.matmul(ps_q, lhsT=wqk_sb[:, kk, 0:128],
                                     rhs=xt[:, kk, :],
                                     start=(kk == 0), stop=False)
                nc.tensor.matmul(ps_q, lhsT=bqk_sb[:, 0:128], rhs=ones_sb[:],
                                 start=False, stop=True)
                for kk in range(KC):
                    nc.tensor.matmul(ps_k, lhsT=wqk_sb[:, kk, 128:256],
                                     rhs=xt[:, kk, :],
                                     start=(kk == 0), stop=False)
                nc.tensor.matmul(ps_k, lhsT=bqk_sb[:, 128:256], rhs=ones_sb[:],
                                 start=False, stop=True)
                qs = slice(sc * 512, (sc + 1) * 512)
                nc.vector.tensor_copy(qT_sb[:, qs], ps_q)
                nc.vector.tensor_copy(kT_sb[:, qs], ps_k)

                for s4 in range(4):
                    sidx = sc * 4 + s4
                    ps_v = psB.tile([128, 512], f32, tag="psB", name="ps_v")[:, :128]
                    for kk in range(KC):
                        nc.tensor.matmul(
                            ps_v,
                            lhsT=xt[:, kk, s4 * 128:(s4 + 1) * 128],
                            rhs=wv_sb[:, kk, :],
                            start=(kk == 0), stop=False)
                    nc.tensor.matmul(ps_v, lhsT=ones_sb[:, 0:128], rhs=bv_sb[:],
                                     start=False, stop=True)
                    nc.vector.tensor_copy(v_sb[:, sidx, 0:64], ps_v[:, 0:64])
                    nc.vector.tensor_copy(v_sb[:, sidx, 65:129], ps_v[:, 64:128])

            emit_qkv_chunk(0)
            scope1.__exit__(None, None, None)
            scope2 = nc.named_scope("attn"); scope2.__enter__()

            # ---- output projection for chunk g (128 rows x D) ----
            def emit_proj(g):
                ctxag_sb = agpool.tile([128, NC, 128], bf16, tag="ctxag",
                                       name="ctxag_sb")
                nc.sync.dma_start(
                    ctxag_sb[:], a2a_out[g].rearrange("(k p) s -> p k s", p=128))
                ot = agpool.tile([128, D], f32, tag="ot")
                for ncol in range(D // 512):
                    ps_o = psB.tile([128, 512], f32, tag="psB", name="ps_o")
                    for k in range(NC):
                        nc.tensor.matmul(
                            ps_o,
                            lhsT=ctxag_sb[:, k, :],
                            rhs=wo_sb[:, k, ncol * 512:(ncol + 1) * 512],
                            start=(k == 0), stop=False)
                    nc.tensor.matmul(
                        ps_o, lhsT=ones_sb[:, 0:128],
                        rhs=bo_sb[:, ncol * 512:(ncol + 1) * 512],
                        start=False, stop=True)
                    nc.vector.tensor_copy(
                        ot[:, ncol * 512:(ncol + 1) * 512], ps_o)
                nc.sync.dma_start(out[g * 128:(g + 1) * 128, :], ot[:])

            pending = []
            # band piece layout: 4 staggered sub-pieces [o*128:512] of the
            # diagonal band, packed at offsets BOFF with widths BWID.
            # Offsets chosen so no matmul output crosses a 512-col PSUM bank.
            BOFF = [0, 512, 1024, 896]
            BWID = [512, 384, 256, 128]
            BTOT = 1280

            def band_ctx_mm(b, hl, j, exp_j, ps_c, o, start, stop, skip=True):
                nr = 4 * j
                nc.tensor.matmul(
                    ps_c[:65, o * 128:512],
                    lhsT=v_sb[:, b * NKT + nr + o, 65 * hl: 65 * hl + 65],
                    rhs=exp_j[:, nr * 512 + BOFF[o]:
                              nr * 512 + BOFF[o] + BWID[o]],
                    start=start, stop=stop, skip_group_check=skip)

            def emit_ctx(b, hl, j, exp_j):
                hp = slice(64 * hl, 64 * hl + 64)
                ps_c = psB.tile([128, 512], f32, tag="psB", name="ps_c")
                nr = 4 * j
                if nr > 0:
                    for tt in range(nr):
                        nc.tensor.matmul(
                            ps_c[:65, :],
                            lhsT=v_sb[:, b * NKT + tt, 65 * hl: 65 * hl + 65],
                            rhs=exp_j[:, tt * 512:(tt + 1) * 512],
                            start=(tt == 0), stop=False)
                    for o in (1, 2, 3):
                        band_ctx_mm(b, hl, j, exp_j, ps_c, o, False, False)
                    # full-width piece last so the group stop covers [0:512]
                    band_ctx_mm(b, hl, j, exp_j, ps_c, 0, False, True,
                                skip=False)
                else:
                    band_ctx_mm(b, hl, j, exp_j, ps_c, 0, True, False,
                                skip=False)
                    for o in (1, 2, 3):
                        band_ctx_mm(b, hl, j, exp_j, ps_c, o, False, False)
                    # zero K=1 matmul closes the group over the full range
                    nc.tensor.matmul(ps_c[:65, :], lhsT=zrow_sb[:],
                                     rhs=ones_sb[:], start=False, stop=True)
                den = spool.tile([1, 512], f32, tag="den")
                nc.vector.tensor_copy(den[:], ps_c[64:65, :])
                recip = spool.tile([1, 512], f32, tag="recip")
                nc.vector.reciprocal_approx_fast(out=recip[:], in_=den[:])
                bcast = spool.tile([128, 512], f32, tag="bcast")
                nc.gpsimd.partition_broadcast(bcast[:], recip[:])
                cs = slice(b * S + j * 512, b * S + (j + 1) * 512)
                nc.vector.tensor_tensor(
                    ctxT_sb[hp, cs], ps_c[0:64, :], bcast[0:64, :], ALU.mult)
                if hl == 1 and j in (1, 3):
                    emit_a2a(2 * b + (j == 3))

            def emit_a2a(g):
                lo = g * 1024
                nc.sync.dma_start(
                    a2a_in[g].rearrange("(d p) c -> p d c", p=128),
                    ctxT_sb[:, lo:lo + 1024].rearrange("p (d c) -> p d c", d=NC))
                nc.gpsimd.collective_compute(
                    "AllToAll",
                    mybir.AluOpType.bypass,
                    replica_groups=[list(range(NC))],
                    ins=[a2a_in[g][:]],
                    outs=[a2a_out[g][:]],
                )

            def flush_pending():
                while pending:
                    emit_ctx(*pending.pop(0))

            # proj chunk emitted at the start of window (b, j) — its AllToAll
            # was launched >=2 windows earlier
            proj_at = {(1, 1): 0, (1, 3): 1}
            # qkv chunks streamed one window ahead of first use
            qkv_at = {(0, 0): [1], (0, 1): [2], (0, 2): [3, 4],
                      (0, 3): [5], (1, 0): [6], (1, 1): [7]}

            for b in range(B):
                for j in range(NQT):
                    for sc_i in qkv_at.get((b, j), []):
                        emit_qkv_chunk(sc_i)
                    if (b, j) in proj_at:
                        emit_proj(proj_at[(b, j)])
                    nr = 4 * j
                    exp_js = []
                    for hl in range(HPC):
                        exp_js.append(epool.tile([128, nr * 512 + BTOT], bf16,
                                                 tag=f"expj{j}h{hl}",
                                                 name="exp_j"))
                    # scores pieces alternate heads so adjacent pieces use
                    # different PE row groups (h0: rows 0:64, h1: 64:128) and
                    # their matmuls execute concurrently in the array.
                    qwins = [qT_sb[slice(64 * hl, 64 * hl + 64),
                                   b * S + j * 512: b * S + (j + 1) * 512]
                             for hl in range(HPC)]
                    tt = 0
                    while tt < nr:
                        npc = min(3, nr - tt)
                        for hl in range(HPC):
                            hp = slice(64 * hl, 64 * hl + 64)
                            ps = psA.tile([128, 1536], f32, tag="psA",
                                          name="ps_sc")[:, :npc * 512]
                            for i in range(npc):
                                kt = b * S + (tt + i) * 128
                                nc.tensor.matmul(
                                    ps[:, i * 512:(i + 1) * 512],
                                    lhsT=kT_sb[hp, kt:kt + 128],
                                    rhs=qwins[hl],
                                    start=True, stop=True)
                            nc.scalar.activation(
                                exp_js[hl][:, tt * 512:(tt + npc) * 512],
                                ps, ACTF.Exp)
                        tt += npc
                    # diagonal band: staggered sub-pieces, one exp per head,
                    # then a [128,128] triangular mask per sub-piece
                    for hl in range(HPC):
                        hp = slice(64 * hl, 64 * hl + 64)
                        ps = psA.tile([128, 1536], f32, tag="psA",
                                      name="ps_band")[:, :BTOT]
                        for o in range(4):
                            kt = b * S + (nr + o) * 128
                            nc.tensor.matmul(
                                ps[:, BOFF[o]:BOFF[o] + BWID[o]],
                                lhsT=kT_sb[hp, kt:kt + 128],
                                rhs=qwins[hl][:, o * 128:512],
                                start=True, stop=True)
                        nc.scalar.activation(
                            exp_js[hl][:, nr * 512:nr * 512 + BTOT],
                            ps, ACTF.Exp)
                        for o in range(4):
                            blk = slice(nr * 512 + BOFF[o],
                                        nr * 512 + BOFF[o] + 128)
                            nc.vector.tensor_tensor(exp_js[hl][:, blk],
                                                    exp_js[hl][:, blk],
                                                    mask_sb[:, 384:512],
                                                    ALU.mult)
                    # lag-2 ctx emission
                    for hl in range(HPC):
                        pending.append((b, hl, j, exp_js[hl]))
                    while len(pending) > 4:
                        emit_ctx(*pending.pop(0))
            flush_pending()
            emit_proj(2)
            emit_proj(3)

            scope2.__exit__(None, None, None)

    nc.compile()
    return nc


def _prep_inputs(x, Wqkv, bqkv, Wo, bo):
    x = np.asarray(x, dtype=np.float32)
    Wqkv = np.asarray(Wqkv, dtype=np.float32)
    bqkv = np.asarray(bqkv, dtype=np.float32)
    Wo = np.asarray(Wo, dtype=np.float32)
    bo = np.asarray(bo, dtype=np.float32)

    xT = np.ascontiguousarray(x.reshape(BS, D).T).astype(BF16)
    wo_b = Wo.astype(BF16)

    kp = np.arange(128)[:, None]
    u = np.arange(896)[None, :]
    mask = (u >= 384 + kp).astype(BF16)

    scale = np.float32(1.0 / np.sqrt(HD))

    # Wqkv columns per head h: q = 192h..+64, k = +64, v = +128
    W3 = Wqkv.reshape(D, H, 3, HD)
    b3 = bqkv.reshape(H, 3, HD)

    in_maps = []
    for c in range(NC):
        hs = [HPC * c + i for i in range(HPC)]
        wq = np.concatenate([W3[:, h, 0, :] for h in hs], axis=1) * scale
        wk = np.concatenate([W3[:, h, 1, :] for h in hs], axis=1)
        wv_ = np.concatenate([W3[:, h, 2, :] for h in hs], axis=1)
        bq = np.concatenate([b3[h, 0, :] for h in hs]) * scale
        bk = np.concatenate([b3[h, 1, :] for h in hs])
        bv_ = np.concatenate([b3[h, 2, :] for h in hs])
        in_maps.append({
            "xT": xT,
            "wqk": np.ascontiguousarray(
                np.concatenate([wq, wk], axis=1)).astype(BF16),
            "wv": np.ascontiguousarray(wv_).astype(BF16),
            "wo": wo_b,
            "bqk": np.concatenate([bq, bk])[None, :].astype(BF16),
            "bv": bv_[None, :].astype(BF16),
            "bo": bo[None, :].astype(BF16),
            "mask": mask,
        })
    return in_maps


def run(x, Wqkv, bqkv, Wo, bo, trace=False):
    from concourse.bass_utils import run_bass_kernel_spmd

    if "nc" not in _CACHE:
        _CACHE["nc"] = _build_program()
    nc = _CACHE["nc"]
    in_maps = _prep_inputs(x, Wqkv, bqkv, Wo, bo)
    res = run_bass_kernel_spmd(nc, in_maps, list(range(NC)), trace=trace)
    # core c returns [512, D]: 4 chunks of 128 rows: (b0 rows 128c..),
    # (b0 rows 1024+128c..), (b1 rows 128c..), (b1 rows 1024+128c..)
    full = np.empty((B, S, D), dtype=np.float32)
    for c in range(NC):
        r = res.results[c]["out"]
        for g in range(4):
            b, half = g // 2, g % 2
            lo = half * 1024 + 128 * c
            full[b, lo:lo + 128, :] = r[g * 128:(g + 1) * 128, :]
    return full, res


def kernel(x, Wqkv, bqkv, Wo, bo):
    out, _ = run(x, Wqkv, bqkv, Wo, bo)
    return out


# revision 15
# speedup vs baseline: 1.2881x; 1.2881x over previous
"""Trainium2 Bass kernel for causal multi-head attention.

Problem: B=2, S=2048, D=1024, H=16 heads (hd=64), fp32 in/out.
  qkv = x @ Wqkv + bqkv ; per-head causal softmax attention ; out = ctx @ Wo + bo

Sharding (8 NeuronCores): tensor-parallel over heads — 2 heads per core.
Each core computes q/k/v projections for its 2 heads (both batches), causal
attention, and its ctx^T slice [128 feat, B*S]. Four AllToAll exchanges
(one per 512-row output chunk) route each core's 128-feature slice of the
other cores' output rows; each core then computes the output projection for
its 512 rows (4 chunks of 128) with the full Wo. Host reassembles.

Schedule: the softmax exp on the Scalar engine paces the scores stream
(0.833ns/elem vs PE's 0.417ns/row-elem), so ACT-independent PE work (the
attn@v of two-windows-ago, qkv projection chunks, output projections) is
woven between score pieces to keep the PE dense and at full p-state clock.

Numerics: bf16 matmul operands, fp32 PSUM accumulation. Softmax uses exp
without max-subtraction (scores ~N(0,1) after the folded 1/sqrt(hd) scale).
The softmax denominator comes free as a ones-column appended to v in the
attn@v matmul. Bias adds are K=1 PE matmuls inside the PSUM accumulation
groups (biases are zero in this problem but kept for fidelity).
"""

import numpy as np
import ml_dtypes

B, S, D, H, NC = 2, 2048, 1024, 16, 8
HD = D // H            # 64
HPC = H // NC          # 2 heads per core
BS = B * S             # 4096
RPB = S // NC          # 256 output rows per core per batch
KC = D // 128          # 8 contraction chunks
SC = BS // 512         # 8 s-chunks for qkv projection
NQT = S // 512         # 4 q-windows (512) per batch
NKT = S // 128         # 16 k-tiles (128) per batch
NCH = 2 * B            # 4 output chunks (b, half)

BF16 = ml_dtypes.bfloat16

_CACHE = {}


def _build_program():
    import concourse.bass as bass
    import concourse.mybir as mybir
    from concourse import bacc
    from concourse.tile import TileContext

    dt = mybir.dt
    f32, bf16 = dt.float32, dt.bfloat16
    ALU = mybir.AluOpType
    ACTF = mybir.ActivationFunctionType

    nc = bacc.Bacc("TRN2", target_bir_lowering=False, debug=False, num_devices=NC)

    xT = nc.dram_tensor("xT", [D, BS], bf16, kind="ExternalInput")
    wqk = nc.dram_tensor("wqk", [D, 256], bf16, kind="ExternalInput")
    wv = nc.dram_tensor("wv", [D, 128], bf16, kind="ExternalInput")
    wo = nc.dram_tensor("wo", [D, D], bf16, kind="ExternalInput")
    bqk = nc.dram_tensor("bqk", [1, 256], bf16, kind="ExternalInput")
    bv = nc.dram_tensor("bv", [1, 128], bf16, kind="ExternalInput")
    bo = nc.dram_tensor("bo", [1, D], bf16, kind="ExternalInput")
    mask = nc.dram_tensor("mask", [128, 896], bf16, kind="ExternalInput")
    out = nc.dram_tensor("out", [NCH * 128, D], f32, kind="ExternalOutput")

    # AllToAll buffers: for chunk g, block d of a2a_in holds my 128 features
    # for destination core d's 128 output rows; a2a_out block s holds core
    # s's 128 features for MY 128 rows of chunk g.
    a2a_in = [nc.dram_tensor(f"a2ain{g}", [NC * 128, 128], bf16)
              for g in range(NCH)]
    a2a_out = [nc.dram_tensor(f"a2aout{g}", [NC * 128, 128], bf16)
               for g in range(NCH)]

    with TileContext(nc) as tc:
        with (
            tc.tile_pool(name="const", bufs=1) as cpool,
            tc.tile_pool(name="big", bufs=1) as bigpool,
            tc.tile_pool(name="xstream", bufs=2) as xpool,
            tc.tile_pool(name="exp", bufs=1) as epool,
            tc.tile_pool(name="small", bufs=3) as spool,
            tc.tile_pool(name="agbuf", bufs=2) as agpool,
            tc.tile_pool(name="psA", bufs=2, space="PSUM") as psA,   # 2x [128,1536]
            tc.tile_pool(name="psB", bufs=2, space="PSUM") as psB,   # 2x [128,512]
        ):
            # ---- urgent constants on the sync DMA queue ----
            wqk_sb = cpool.tile([128, KC, 256], bf16, tag="wqk")
            nc.sync.dma_start(wqk_sb[:], wqk.rearrange("(ko p) m -> p ko m", p=128))
            wv_sb = cpool.tile([128, KC, 128], bf16, tag="wv")
            nc.sync.dma_start(wv_sb[:], wv.rearrange("(ko p) m -> p ko m", p=128))
            bqk_sb = cpool.tile([1, 256], bf16, tag="bqk")
            nc.sync.dma_start(bqk_sb[:], bqk[:])
            bv_sb = cpool.tile([1, 128], bf16, tag="bv")
            nc.sync.dma_start(bv_sb[:], bv[:])
            # ---- lazy constants on the gpsimd DMA queue (not needed until
            # masks/proj, keeps the sync queue clear for x streaming) ----
            wo_sb = cpool.tile([128, KC, D], bf16, tag="wo")
            nc.gpsimd.dma_start(wo_sb[:], wo.rearrange("(ko p) m -> p ko m", p=128))
            bo_sb = cpool.tile([1, D], bf16, tag="bo")
            nc.gpsimd.dma_start(bo_sb[:], bo[:])
            mask_sb = cpool.tile([128, 896], bf16, tag="mask")
            nc.gpsimd.dma_start(mask_sb[:], mask[:])

            ones_sb = cpool.tile([1, 512], bf16, tag="ones")
            nc.vector.memset(ones_sb[:], 1.0)
            zrow_sb = cpool.tile([1, 65], bf16, tag="zrow")
            nc.vector.memset(zrow_sb[:], 0.0)

            # ---- persistent activations ----
            qT_sb = bigpool.tile([128, BS], bf16, tag="qT")   # [2*64 feat, B*S]
            kT_sb = bigpool.tile([128, BS], bf16, tag="kT")
            # v natural layout + ones cols: per 128-row chunk:
            #   [v_h0(0:64) | ones(64) | v_h1(65:129) | ones(129)]
            v_sb = bigpool.tile([128, BS // 128, 130], bf16, tag="v")
            ctxT_sb = bigpool.tile([128, BS], bf16, tag="ctxT")

            nc.vector.memset(v_sb[:, :, 64:65], 1.0)
            nc.vector.memset(v_sb[:, :, 129:130], 1.0)

            xT_r = xT.rearrange("(ko p) s -> p ko s", p=128)

            # ---- qkv projection chunk, split into PE-filler steps ----
            def qkv_steps(sc):
                xt = xpool.tile([128, KC, 512], bf16, tag="xt")
                nc.sync.dma_start(xt[:], xT_r[:, :, sc * 512:(sc + 1) * 512])
                qs = slice(sc * 512, (sc + 1) * 512)

                def qk_step(col0, dst):
                    ps = psA.tile([128, 1536], f32, tag="psA",
                                  name="ps_qk")[:, :512]
                    for kk in range(KC):
                        nc.tensor.matmul(ps, lhsT=wqk_sb[:, kk, col0:col0 + 128],
                                         rhs=xt[:, kk, :],
                                         start=(kk == 0), stop=False)
                    nc.tensor.matmul(ps, lhsT=bqk_sb[:, col0:col0 + 128],
                                     rhs=ones_sb[:], start=False, stop=True)
                    nc.vector.tensor_copy(dst[:, qs], ps)

                def v_step(s4):
                    sidx = sc * 4 + s4
                    ps_v = psB.tile([128, 512], f32, tag="psB",
                                    name="ps_v")[:, :128]
                    for kk in range(KC):
                        nc.tensor.matmul(
                            ps_v,
                            lhsT=xt[:, kk, s4 * 128:(s4 + 1) * 128],
                            rhs=wv_sb[:, kk, :],
                            start=(kk == 0), stop=False)
                    nc.tensor.matmul(ps_v, lhsT=ones_sb[:, 0:128], rhs=bv_sb[:],
                                     start=False, stop=True)
                    nc.vector.tensor_copy(v_sb[:, sidx, 0:64], ps_v[:, 0:64])
                    nc.vector.tensor_copy(v_sb[:, sidx, 65:129], ps_v[:, 64:128])

                return [lambda: qk_step(0, qT_sb), lambda: qk_step(128, kT_sb),
                        lambda: (v_step(0), v_step(1)),
                        lambda: (v_step(2), v_step(3))]

            # ---- output projection for chunk g, split into 2 steps ----
            def proj_steps(g):
                ctxag_sb = agpool.tile([128, NC, 128], bf16, tag="ctxag",
                                       name="ctxag_sb")
                nc.sync.dma_start(
                    ctxag_sb[:], a2a_out[g].rearrange("(k p) s -> p k s", p=128))
                ot = agpool.tile([128, D], f32, tag="ot")

                def ncol_step(ncol, last):
                    ps_o = psB.tile([128, 512], f32, tag="psB", name="ps_o")
                    for k in range(NC):
                        nc.tensor.matmul(
                            ps_o,
                            lhsT=ctxag_sb[:, k, :],
                            rhs=wo_sb[:, k, ncol * 512:(ncol + 1) * 512],
                            start=(k == 0), stop=False)
                    nc.tensor.matmul(
                        ps_o, lhsT=ones_sb[:, 0:128],
                        rhs=bo_sb[:, ncol * 512:(ncol + 1) * 512],
                        start=False, stop=True)
                    nc.vector.tensor_copy(
                        ot[:, ncol * 512:(ncol + 1) * 512], ps_o)
                    if last:
                        nc.sync.dma_start(out[g * 128:(g + 1) * 128, :], ot[:])

                return [lambda: ncol_step(0, False), lambda: ncol_step(1, True)]

            # band piece layout: 4 staggered sub-pieces [o*128:512] of the
            # diagonal band, packed at offsets BOFF with widths BWID.
            # Offsets chosen so no matmul output crosses a 512-col PSUM bank.
            BOFF = [0, 512, 1024, 896]
            BWID = [512, 384, 256, 128]
            BTOT = 1280

            # ---- attn@v for one (batch, head, window), as filler steps ----
            def ctx_steps(b, hl, j, exp_j):
                hp = slice(64 * hl, 64 * hl + 64)
                nr = 4 * j
                state = {}

                def band_mm(ps_c, o, start, stop, skip=True):
                    nc.tensor.matmul(
                        ps_c[:65, o * 128:512],
                        lhsT=v_sb[:, b * NKT + nr + o, 65 * hl: 65 * hl + 65],
                        rhs=exp_j[:, nr * 512 + BOFF[o]:
                                  nr * 512 + BOFF[o] + BWID[o]],
                        start=start, stop=stop, skip_group_check=skip)

                def mm_run(lo, hi):
                    if lo == 0:
                        state["ps_c"] = psB.tile([128, 512], f32, tag="psB",
                                                 name="ps_c")
                    ps_c = state["ps_c"]
                    for i in range(lo, hi):
                        if nr > 0:
                            # order: rect tiles, bands 1..3, band 0 last
                            if i < nr:
                                nc.tensor.matmul(
                                    ps_c[:65, :],
                                    lhsT=v_sb[:, b * NKT + i,
                                              65 * hl: 65 * hl + 65],
                                    rhs=exp_j[:, i * 512:(i + 1) * 512],
                                    start=(i == 0), stop=False,
                                    skip_group_check=(i > 0))
                            elif i < nr + 3:
                                band_mm(ps_c, i - nr + 1, False, False)
                            else:
                                band_mm(ps_c, 0, False, True, skip=False)
                        else:
                            # j == 0: band 0 first, then 1..3, then closer
                            if i == 0:
                                band_mm(ps_c, 0, True, False, skip=False)
                            elif i < 4:
                                band_mm(ps_c, i, False, False)
                            else:
                                nc.tensor.matmul(
                                    ps_c[:65, :], lhsT=zrow_sb[:],
                                    rhs=ones_sb[:], start=False, stop=True)

                def finalize():
                    ps_c = state["ps_c"]
                    den = spool.tile([1, 512], f32, tag="den")
                    nc.vector.tensor_copy(den[:], ps_c[64:65, :])
                    recip = spool.tile([1, 512], f32, tag="recip")
                    nc.vector.reciprocal_approx_fast(out=recip[:], in_=den[:])
                    bcast = spool.tile([128, 512], f32, tag="bcast")
                    nc.gpsimd.partition_broadcast(bcast[:], recip[:])
                    cs = slice(b * S + j * 512, b * S + (j + 1) * 512)
                    nc.vector.tensor_tensor(
                        ctxT_sb[hp, cs], ps_c[0:64, :], bcast[0:64, :],
                        ALU.mult)
                    if hl == 1 and j in (1, 3):
                        emit_a2a(2 * b + (j == 3))

                nmm = (nr + 4) if nr > 0 else 5
                steps = []
                lo = 0
                while lo < nmm:
                    hi = min(lo + 5, nmm)
                    steps.append(lambda lo=lo, hi=hi: mm_run(lo, hi))
                    lo = hi
                steps.append(finalize)
                return steps

            def emit_a2a(g):
                lo = g * 1024
                nc.sync.dma_start(
                    a2a_in[g].rearrange("(d p) c -> p d c", p=128),
                    ctxT_sb[:, lo:lo + 1024].rearrange("p (d c) -> p d c", d=NC))
                nc.gpsimd.collective_compute(
                    "AllToAll",
                    mybir.AluOpType.bypass,
                    replica_groups=[list(range(NC))],
                    ins=[a2a_in[g][:]],
                    outs=[a2a_out[g][:]],
                )

            # ---- schedule ----
            pending = []
            proj_at = {(1, 1): 0, (1, 3): 1}
            qkv_at = {(0, 0): [1], (0, 1): [2], (0, 2): [3, 4],
                      (0, 3): [5], (1, 0): [6], (1, 1): [7]}
            pop_to = {(0, 2): 2}     # fire chunk-0 A2A one window early

            scope1 = nc.named_scope("qkv"); scope1.__enter__()
            for st in qkv_steps(0):
                st()
            scope1.__exit__(None, None, None)

            for b in range(B):
                for j in range(NQT):
                    scope = nc.named_scope(f"w{b}{j}"); scope.__enter__()
                    nr = 4 * j
                    exp_js = [epool.tile([128, nr * 512 + BTOT], bf16,
                                         tag=f"expj{j}h{hl}", name="exp_j")
                              for hl in range(HPC)]
                    qwins = [qT_sb[slice(64 * hl, 64 * hl + 64),
                                   b * S + j * 512: b * S + (j + 1) * 512]
                             for hl in range(HPC)]

                    # scores pieces (ACT-paced): rect pieces then band pieces
                    def rect_piece(hl, tt, npc):
                        hp = slice(64 * hl, 64 * hl + 64)
                        ps = psA.tile([128, 1536], f32, tag="psA",
                                      name="ps_sc")[:, :npc * 512]
                        for i in range(npc):
                            kt = b * S + (tt + i) * 128
                            nc.tensor.matmul(
                                ps[:, i * 512:(i + 1) * 512],
                                lhsT=kT_sb[hp, kt:kt + 128],
                                rhs=qwins[hl],
                                start=True, stop=True)
                        nc.scalar.activation(
                            exp_js[hl][:, tt * 512:(tt + npc) * 512],
                            ps, ACTF.Exp)

                    def band_piece(hl):
                        hp = slice(64 * hl, 64 * hl + 64)
                        ps = psA.tile([128, 1536], f32, tag="psA",
                                      name="ps_band")[:, :BTOT]
                        for o in range(4):
                            kt = b * S + (nr + o) * 128
                            nc.tensor.matmul(
                                ps[:, BOFF[o]:BOFF[o] + BWID[o]],
                                lhsT=kT_sb[hp, kt:kt + 128],
                                rhs=qwins[hl][:, o * 128:512],
                                start=True, stop=True)
                        nc.scalar.activation(
                            exp_js[hl][:, nr * 512:nr * 512 + BTOT],
                            ps, ACTF.Exp)
                        for o in range(4):
                            blk = slice(nr * 512 + BOFF[o],
                                        nr * 512 + BOFF[o] + 128)
                            nc.vector.tensor_tensor(exp_js[hl][:, blk],
                                                    exp_js[hl][:, blk],
                                                    mask_sb[:, 384:512],
                                                    ALU.mult)

                    pieces = []
                    tt = 0
                    while tt < nr:
                        npc = min(3, nr - tt)
                        for hl in range(HPC):
                            pieces.append(
                                lambda hl=hl, tt=tt, npc=npc:
                                rect_piece(hl, tt, npc))
                        tt += npc
                    for hl in range(HPC):
                        pieces.append(lambda hl=hl: band_piece(hl))

                    # ACT-independent PE fillers
                    fillers = []
                    for sc_i in qkv_at.get((b, j), []):
                        fillers += qkv_steps(sc_i)
                    if (b, j) in proj_at:
                        fillers += proj_steps(proj_at[(b, j)])
                    for hl in range(HPC):
                        pending.append((b, hl, j, exp_js[hl]))
                    thresh = pop_to.get((b, j), 4)
                    while len(pending) > thresh:
                        fillers += ctx_steps(*pending.pop(0))

                    # weave: scores pieces paced by ACT, fillers between
                    fi = 0
                    for pi, piece in enumerate(pieces):
                        piece()
                        rem_p = len(pieces) - pi - 1
                        want = ((len(fillers) - fi) + rem_p) // (rem_p + 1)
                        for _ in range(want):
                            fillers[fi](); fi += 1
                    while fi < len(fillers):
                        fillers[fi](); fi += 1
                    scope.__exit__(None, None, None)

            scope3 = nc.named_scope("tail"); scope3.__enter__()
            while pending:
                for st in ctx_steps(*pending.pop(0)):
                    st()
            for st in proj_steps(2) + proj_steps(3):
                st()
            scope3.__exit__(None, None, None)

    nc.compile()
    return nc


def _prep_inputs(x, Wqkv, bqkv, Wo, bo):
    x = np.asarray(x, dtype=np.float32)
    Wqkv = np.asarray(Wqkv, dtype=np.float32)
    bqkv = np.asarray(bqkv, dtype=np.float32)
    Wo = np.asarray(Wo, dtype=np.float32)
    bo = np.asarray(bo, dtype=np.float32)

    xT = np.ascontiguousarray(x.reshape(BS, D).T).astype(BF16)
    wo_b = Wo.astype(BF16)

    kp = np.arange(128)[:, None]
    u = np.arange(896)[None, :]
    mask = (u >= 384 + kp).astype(BF16)

    scale = np.float32(1.0 / np.sqrt(HD))

    # Wqkv columns per head h: q = 192h..+64, k = +64, v = +128
    W3 = Wqkv.reshape(D, H, 3, HD)
    b3 = bqkv.reshape(H, 3, HD)

    in_maps = []
    for c in range(NC):
        hs = [HPC * c + i for i in range(HPC)]
        wq = np.concatenate([W3[:, h, 0, :] for h in hs], axis=1) * scale
        wk = np.concatenate([W3[:, h, 1, :] for h in hs], axis=1)
        wv_ = np.concatenate([W3[:, h, 2, :] for h in hs], axis=1)
        bq = np.concatenate([b3[h, 0, :] for h in hs]) * scale
        bk = np.concatenate([b3[h, 1, :] for h in hs])
        bv_ = np.concatenate([b3[h, 2, :] for h in hs])
        in_maps.append({
            "xT": xT,
            "wqk": np.ascontiguousarray(
                np.concatenate([wq, wk], axis=1)).astype(BF16),
            "wv": np.ascontiguousarray(wv_).astype(BF16),
            "wo": wo_b,
            "bqk": np.concatenate([bq, bk])[None, :].astype(BF16),
            "bv": bv_[None, :].astype(BF16),
            "bo": bo[None, :].astype(BF16),
            "mask": mask,
        })
    return in_maps


def run(x, Wqkv, bqkv, Wo, bo, trace=False):
    from concourse.bass_utils import run_bass_kernel_spmd

    if "nc" not in _CACHE:
        _CACHE["nc"] = _build_program()
    nc = _CACHE["nc"]
    in_maps = _prep_inputs(x, Wqkv, bqkv, Wo, bo)
    res = run_bass_kernel_spmd(nc, in_maps, list(range(NC)), trace=trace)
    # core c returns [512, D]: 4 chunks of 128 rows: (b0 rows 128c..),
    # (b0 rows 1024+128c..), (b1 rows 128c..), (b1 rows 1024+128c..)
    full = np.empty((B, S, D), dtype=np.float32)
    for c in range(NC):
        r = res.results[c]["out"]
        for g in range(4):
            b, half = g // 2, g % 2
            lo = half * 1024 + 128 * c
            full[b, lo:lo + 128, :] = r[g * 128:(g + 1) * 128, :]
    return full, res


def kernel(x, Wqkv, bqkv, Wo, bo):
    out, _ = run(x, Wqkv, bqkv, Wo, bo)
    return out


# revision 24
# speedup vs baseline: 1.3305x; 1.0329x over previous
"""Trainium2 Bass kernel for causal multi-head attention.

Problem: B=2, S=2048, D=1024, H=16 heads (hd=64), fp32 in/out.
  qkv = x @ Wqkv + bqkv ; per-head causal softmax attention ; out = ctx @ Wo + bo

Sharding (8 NeuronCores): tensor-parallel over heads — 2 heads per core.
Each core computes q/k/v projections for its 2 heads (both batches), causal
attention, and its ctx^T slice [128 feat, B*S]. Four AllToAll exchanges
(one per 512-row output chunk) route each core's 128-feature slice of the
other cores' output rows; each core then computes the output projection for
its 512 rows (4 chunks of 128) with the full Wo. Host reassembles.

Schedule: the softmax exp on the Scalar engine paces the scores stream
(0.833ns/elem vs PE's 0.417ns/row-elem), so ACT-independent PE work (the
attn@v of two-windows-ago, qkv projection chunks, output projections) is
woven between score pieces to keep the PE dense and at full p-state clock.

Numerics: bf16 matmul operands, fp32 PSUM accumulation. Softmax uses exp
without max-subtraction (scores ~N(0,1) after the folded 1/sqrt(hd) scale).
The softmax denominator comes free as a ones-column appended to v in the
attn@v matmul. Bias adds are K=1 PE matmuls inside the PSUM accumulation
groups (biases are zero in this problem but kept for fidelity).
"""

import numpy as np
import ml_dtypes

B, S, D, H, NC = 2, 2048, 1024, 16, 8
HD = D // H            # 64
HPC = H // NC          # 2 heads per core
BS = B * S             # 4096
RPB = S // NC          # 256 output rows per core per batch
KC = D // 128          # 8 contraction chunks
SC = BS // 512         # 8 s-chunks for qkv projection
NQT = S // 512         # 4 q-windows (512) per batch
NKT = S // 128         # 16 k-tiles (128) per batch
NCH = 2 * B            # 4 output chunks (b, half)

BF16 = ml_dtypes.bfloat16

_CACHE = {}


def _build_program():
    import concourse.bass as bass
    import concourse.mybir as mybir
    from concourse import bacc
    from concourse.tile import TileContext

    dt = mybir.dt
    f32, bf16 = dt.float32, dt.bfloat16
    ALU = mybir.AluOpType
    ACTF = mybir.ActivationFunctionType

    nc = bacc.Bacc("TRN2", target_bir_lowering=False, debug=False, num_devices=NC)

    xT = nc.dram_tensor("xT", [D, BS], bf16, kind="ExternalInput")
    wqk = nc.dram_tensor("wqk", [D, 256], bf16, kind="ExternalInput")
    wv = nc.dram_tensor("wv", [D, 128], bf16, kind="ExternalInput")
    wo = nc.dram_tensor("wo", [D, D], bf16, kind="ExternalInput")
    bqk = nc.dram_tensor("bqk", [1, 256], bf16, kind="ExternalInput")
    bv = nc.dram_tensor("bv", [1, 128], bf16, kind="ExternalInput")
    bo = nc.dram_tensor("bo", [1, D], bf16, kind="ExternalInput")
    mask = nc.dram_tensor("mask", [128, 896], bf16, kind="ExternalInput")
    out = nc.dram_tensor("out", [NCH * 128, D], f32, kind="ExternalOutput")

    # AllToAll buffers: for chunk g, block d of a2a_in holds my 128 features
    # for destination core d's 128 output rows; a2a_out block s holds core
    # s's 128 features for MY 128 rows of chunk g.
    a2a_in = [nc.dram_tensor(f"a2ain{g}", [NC * 128, 128], bf16)
              for g in range(NCH)]
    a2a_out = [nc.dram_tensor(f"a2aout{g}", [NC * 128, 128], bf16)
               for g in range(NCH)]

    with TileContext(nc) as tc:
        with (
            tc.tile_pool(name="const", bufs=1) as cpool,
            tc.tile_pool(name="big", bufs=1) as bigpool,
            tc.tile_pool(name="xstream", bufs=3) as xpool,
            tc.tile_pool(name="exp", bufs=1) as epool,
            tc.tile_pool(name="small", bufs=3) as spool,
            tc.tile_pool(name="agbuf", bufs=2) as agpool,
            tc.tile_pool(name="psA", bufs=2, space="PSUM") as psA,   # 2x [128,1536]
            tc.tile_pool(name="psB", bufs=2, space="PSUM") as psB,   # 2x [128,512]
        ):
            # ---- urgent constants on the sync DMA queue ----
            wqk_sb = cpool.tile([128, KC, 256], bf16, tag="wqk")
            nc.sync.dma_start(wqk_sb[:], wqk.rearrange("(ko p) m -> p ko m", p=128))
            wv_sb = cpool.tile([128, KC, 128], bf16, tag="wv")
            nc.sync.dma_start(wv_sb[:], wv.rearrange("(ko p) m -> p ko m", p=128))
            bqk_sb = cpool.tile([1, 256], bf16, tag="bqk")
            nc.sync.dma_start(bqk_sb[:], bqk[:])
            bv_sb = cpool.tile([1, 128], bf16, tag="bv")
            nc.sync.dma_start(bv_sb[:], bv[:])
            # ---- lazy constants on the gpsimd DMA queue (not needed until
            # masks/proj, keeps the sync queue clear for x streaming) ----
            mask_sb = cpool.tile([128, 896], bf16, tag="mask")
            nc.gpsimd.dma_start(mask_sb[:], mask[:])
            bo_sb = cpool.tile([1, D], bf16, tag="bo")
            nc.gpsimd.dma_start(bo_sb[:], bo[:])
            wo_sb = cpool.tile([128, KC, D], bf16, tag="wo")
            nc.gpsimd.dma_start(wo_sb[:], wo.rearrange("(ko p) m -> p ko m", p=128))

            ones_sb = cpool.tile([1, 512], bf16, tag="ones")
            nc.vector.memset(ones_sb[:], 1.0)
            zrow_sb = cpool.tile([1, 65], bf16, tag="zrow")
            nc.vector.memset(zrow_sb[:], 0.0)

            # ---- persistent activations ----
            qT_sb = bigpool.tile([128, BS], bf16, tag="qT")   # [2*64 feat, B*S]
            kT_sb = bigpool.tile([128, BS], bf16, tag="kT")
            # v natural layout + ones cols: per 128-row chunk:
            #   [v_h0(0:64) | ones(64) | v_h1(65:129) | ones(129)]
            v_sb = bigpool.tile([128, BS // 128, 130], bf16, tag="v")
            ctxT_sb = bigpool.tile([128, BS], bf16, tag="ctxT")

            nc.vector.memset(v_sb[:, :, 64:65], 1.0)
            nc.vector.memset(v_sb[:, :, 129:130], 1.0)

            xT_r = xT.rearrange("(ko p) s -> p ko s", p=128)

            # x-chunk stream: DMAs all issued upfront on the dedicated sync
            # queue; each waits only for its ring buffer to free (bufs=3)
            xts = {}

            def xt_fetch(sc):
                xts[sc] = xpool.tile([128, KC, 512], bf16, tag="xt",
                                     name="xt")
                nc.sync.dma_start(xts[sc][:],
                                  xT_r[:, :, sc * 512:(sc + 1) * 512])

            # ---- qkv projection chunk, split into PE-filler steps ----
            def qkv_steps(sc):
                xt = xts[sc]
                qs = slice(sc * 512, (sc + 1) * 512)

                def qk_step(col0, dst):
                    ps = psA.tile([128, 1536], f32, tag="psA",
                                  name="ps_qk")[:, :512]
                    for kk in range(KC):
                        nc.tensor.matmul(ps, lhsT=wqk_sb[:, kk, col0:col0 + 128],
                                         rhs=xt[:, kk, :],
                                         start=(kk == 0), stop=False)
                    nc.tensor.matmul(ps, lhsT=bqk_sb[:, col0:col0 + 128],
                                     rhs=ones_sb[:], start=False, stop=True)
                    nc.vector.tensor_copy(dst[:, qs], ps)

                def v_step(s4):
                    sidx = sc * 4 + s4
                    ps_v = psB.tile([128, 512], f32, tag="psB",
                                    name="ps_v")[:, :128]
                    for kk in range(KC):
                        nc.tensor.matmul(
                            ps_v,
                            lhsT=xt[:, kk, s4 * 128:(s4 + 1) * 128],
                            rhs=wv_sb[:, kk, :],
                            start=(kk == 0), stop=False)
                    nc.tensor.matmul(ps_v, lhsT=ones_sb[:, 0:128], rhs=bv_sb[:],
                                     start=False, stop=True)
                    nc.vector.tensor_copy(v_sb[:, sidx, 0:64], ps_v[:, 0:64])
                    nc.vector.tensor_copy(v_sb[:, sidx, 65:129], ps_v[:, 64:128])

                return [lambda: qk_step(0, qT_sb), lambda: qk_step(128, kT_sb),
                        lambda: (v_step(0), v_step(1)),
                        lambda: (v_step(2), v_step(3))]

            # ---- output projection for chunk g, split into 2 steps ----
            def proj_steps(g):
                ctxag_sb = agpool.tile([128, NC, 128], bf16, tag="ctxag",
                                       name="ctxag_sb")
                nc.scalar.dma_start(
                    ctxag_sb[:], a2a_out[g].rearrange("(k p) s -> p k s", p=128))
                ot = agpool.tile([128, D], f32, tag="ot")

                def ncol_step(ncol, last):
                    ps_o = psB.tile([128, 512], f32, tag="psB", name="ps_o")
                    for k in range(NC):
                        nc.tensor.matmul(
                            ps_o,
                            lhsT=ctxag_sb[:, k, :],
                            rhs=wo_sb[:, k, ncol * 512:(ncol + 1) * 512],
                            start=(k == 0), stop=False)
                    nc.tensor.matmul(
                        ps_o, lhsT=ones_sb[:, 0:128],
                        rhs=bo_sb[:, ncol * 512:(ncol + 1) * 512],
                        start=False, stop=True)
                    nc.vector.tensor_copy(
                        ot[:, ncol * 512:(ncol + 1) * 512], ps_o)
                    if last:
                        nc.scalar.dma_start(out[g * 128:(g + 1) * 128, :],
                                            ot[:])

                return [lambda: ncol_step(0, False), lambda: ncol_step(1, True)]

            # band piece layout: 4 staggered sub-pieces [o*128:512] of the
            # diagonal band, packed at offsets BOFF with widths BWID.
            # Offsets chosen so no matmul output crosses a 512-col PSUM bank.
            BOFF = [0, 512, 1024, 896]
            BWID = [512, 384, 256, 128]
            BTOT = 1280

            # ---- attn@v for one (batch, head, window), as filler steps ----
            def ctx_steps(b, hl, j, exp_j):
                hp = slice(64 * hl, 64 * hl + 64)
                nr = 4 * j
                state = {}

                def band_mm(ps_c, o, start, stop, skip=True):
                    nc.tensor.matmul(
                        ps_c[:65, o * 128:512],
                        lhsT=v_sb[:, b * NKT + nr + o, 65 * hl: 65 * hl + 65],
                        rhs=exp_j[:, nr * 512 + BOFF[o]:
                                  nr * 512 + BOFF[o] + BWID[o]],
                        start=start, stop=stop, skip_group_check=skip)

                def mm_run(lo, hi):
                    if lo == 0:
                        state["ps_c"] = psB.tile([128, 512], f32, tag="psB",
                                                 name="ps_c")
                    ps_c = state["ps_c"]
                    for i in range(lo, hi):
                        if nr > 0:
                            # order: rect tiles, bands 1..3, band 0 last
                            if i < nr:
                                nc.tensor.matmul(
                                    ps_c[:65, :],
                                    lhsT=v_sb[:, b * NKT + i,
                                              65 * hl: 65 * hl + 65],
                                    rhs=exp_j[:, i * 512:(i + 1) * 512],
                                    start=(i == 0), stop=False,
                                    skip_group_check=(i > 0))
                            elif i < nr + 3:
                                band_mm(ps_c, i - nr + 1, False, False)
                            else:
                                band_mm(ps_c, 0, False, True, skip=False)
                        else:
                            # j == 0: band 0 first, then 1..3, then closer
                            if i == 0:
                                band_mm(ps_c, 0, True, False, skip=False)
                            elif i < 4:
                                band_mm(ps_c, i, False, False)
                            else:
                                nc.tensor.matmul(
                                    ps_c[:65, :], lhsT=zrow_sb[:],
                                    rhs=ones_sb[:], start=False, stop=True)

                def finalize():
                    ps_c = state["ps_c"]
                    den = spool.tile([1, 512], f32, tag="den")
                    nc.vector.tensor_copy(den[:], ps_c[64:65, :])
                    recip = spool.tile([1, 512], f32, tag="recip")
                    nc.vector.reciprocal_approx_fast(out=recip[:], in_=den[:])
                    bcast = spool.tile([128, 512], f32, tag="bcast")
                    nc.gpsimd.partition_broadcast(bcast[:], recip[:])
                    cs = slice(b * S + j * 512, b * S + (j + 1) * 512)
                    nc.vector.tensor_tensor(
                        ctxT_sb[hp, cs], ps_c[0:64, :], bcast[0:64, :],
                        ALU.mult)
                    if hl == 1 and j in (1, 3):
                        emit_a2a(2 * b + (j == 3))

                nmm = (nr + 4) if nr > 0 else 5
                steps = []
                lo = 0
                while lo < nmm:
                    hi = min(lo + 5, nmm)
                    steps.append(lambda lo=lo, hi=hi: mm_run(lo, hi))
                    lo = hi
                steps.append(finalize)
                return steps

            def emit_a2a(g):
                lo = g * 1024
                nc.scalar.dma_start(
                    a2a_in[g].rearrange("(d p) c -> p d c", p=128),
                    ctxT_sb[:, lo:lo + 1024].rearrange("p (d c) -> p d c", d=NC))
                nc.gpsimd.collective_compute(
                    "AllToAll",
                    mybir.AluOpType.bypass,
                    replica_groups=[list(range(NC))],
                    ins=[a2a_in[g][:]],
                    outs=[a2a_out[g][:]],
                )

            # ---- schedule ----
            pending = []
            proj_at = {(1, 1): 0, (1, 3): 1}
            qkv_at = {(0, 0): [1], (0, 1): [2], (0, 2): [3, 4],
                      (0, 3): [5], (1, 0): [6], (1, 1): [7]}
            # fire chunk-0 A2A one window early; drain the j2 ctx units
            # inside window (1,3) to shorten the tail
            pop_to = {(0, 2): 2, (1, 3): 2}

            scope1 = nc.named_scope("qkv"); scope1.__enter__()
            xt_fetch(0)
            for st in qkv_steps(0):
                st()
            for sc_i in range(1, SC):
                xt_fetch(sc_i)
            scope1.__exit__(None, None, None)

            for b in range(B):
                for j in range(NQT):
                    scope = nc.named_scope(f"w{b}{j}"); scope.__enter__()
                    nr = 4 * j
                    exp_js = [epool.tile([128, nr * 512 + BTOT], bf16,
                                         tag=f"expj{j}h{hl}", name="exp_j")
                              for hl in range(HPC)]
                    qwins = [qT_sb[slice(64 * hl, 64 * hl + 64),
                                   b * S + j * 512: b * S + (j + 1) * 512]
                             for hl in range(HPC)]

                    # scores pieces (ACT-paced): rect pieces then band pieces
                    def rect_piece(hl, tt, npc):
                        hp = slice(64 * hl, 64 * hl + 64)
                        ps = psA.tile([128, 1536], f32, tag="psA",
                                      name="ps_sc")[:, :npc * 512]
                        for i in range(npc):
                            kt = b * S + (tt + i) * 128
                            nc.tensor.matmul(
                                ps[:, i * 512:(i + 1) * 512],
                                lhsT=kT_sb[hp, kt:kt + 128],
                                rhs=qwins[hl],
                                start=True, stop=True)
                        nc.scalar.activation(
                            exp_js[hl][:, tt * 512:(tt + npc) * 512],
                            ps, ACTF.Exp)

                    def band_piece(hl):
                        hp = slice(64 * hl, 64 * hl + 64)
                        ps = psA.tile([128, 1536], f32, tag="psA",
                                      name="ps_band")[:, :BTOT]
                        for o in range(4):
                            kt = b * S + (nr + o) * 128
                            nc.tensor.matmul(
                                ps[:, BOFF[o]:BOFF[o] + BWID[o]],
                                lhsT=kT_sb[hp, kt:kt + 128],
                                rhs=qwins[hl][:, o * 128:512],
                                start=True, stop=True)
                        nc.scalar.activation(
                            exp_js[hl][:, nr * 512:nr * 512 + BTOT],
                            ps, ACTF.Exp)
                        for o in range(4):
                            blk = slice(nr * 512 + BOFF[o],
                                        nr * 512 + BOFF[o] + 128)
                            nc.vector.tensor_tensor(exp_js[hl][:, blk],
                                                    exp_js[hl][:, blk],
                                                    mask_sb[:, 384:512],
                                                    ALU.mult)

                    pieces = []
                    tt = 0
                    while tt < nr:
                        npc = min(3, nr - tt)
                        for hl in range(HPC):
                            pieces.append(
                                lambda hl=hl, tt=tt, npc=npc:
                                rect_piece(hl, tt, npc))
                        tt += npc
                    for hl in range(HPC):
                        pieces.append(lambda hl=hl: band_piece(hl))

                    # ACT-independent PE fillers
                    fillers = []
                    for sc_i in qkv_at.get((b, j), []):
                        fillers += qkv_steps(sc_i)
                    if (b, j) in proj_at:
                        fillers += proj_steps(proj_at[(b, j)])
                    for hl in range(HPC):
                        pending.append((b, hl, j, exp_js[hl]))
                    thresh = pop_to.get((b, j), 4)
                    while len(pending) > thresh:
                        fillers += ctx_steps(*pending.pop(0))

                    # weave: scores pieces paced by ACT, fillers between
                    fi = 0
                    for pi, piece in enumerate(pieces):
                        piece()
                        rem_p = len(pieces) - pi - 1
                        want = ((len(fillers) - fi) + rem_p) // (rem_p + 1)
                        for _ in range(want):
                            fillers[fi](); fi += 1
                    while fi < len(fillers):
                        fillers[fi](); fi += 1
                    scope.__exit__(None, None, None)

            scope3 = nc.named_scope("tail"); scope3.__enter__()
            # interleave the remaining ctx units pairwise so the PE keeps
            # matmuls in flight while each unit's normalize chain drains
            while pending:
                pair = [ctx_steps(*pending.pop(0))]
                if pending:
                    pair.append(ctx_steps(*pending.pop(0)))
                n = max(len(p) for p in pair)
                for i in range(n):
                    for p in pair:
                        if i < len(p):
                            p[i]()
            for st in proj_steps(2) + proj_steps(3):
                st()
            scope3.__exit__(None, None, None)

    nc.compile()
    return nc


def _prep_inputs(x, Wqkv, bqkv, Wo, bo):
    x = np.asarray(x, dtype=np.float32)
    Wqkv = np.asarray(Wqkv, dtype=np.float32)
    bqkv = np.asarray(bqkv, dtype=np.float32)
    Wo = np.asarray(Wo, dtype=np.float32)
    bo = np.asarray(bo, dtype=np.float32)

    xT = np.ascontiguousarray(x.reshape(BS, D).T).astype(BF16)
    wo_b = Wo.astype(BF16)

    kp = np.arange(128)[:, None]
    u = np.arange(896)[None, :]
    mask = (u >= 384 + kp).astype(BF16)

    scale = np.float32(1.0 / np.sqrt(HD))

    # Wqkv columns per head h: q = 192h..+64, k = +64, v = +128
    W3 = Wqkv.reshape(D, H, 3, HD)
    b3 = bqkv.reshape(H, 3, HD)

    in_maps = []
    for c in range(NC):
        hs = [HPC * c + i for i in range(HPC)]
        wq = np.concatenate([W3[:, h, 0, :] for h in hs], axis=1) * scale
        wk = np.concatenate([W3[:, h, 1, :] for h in hs], axis=1)
        wv_ = np.concatenate([W3[:, h, 2, :] for h in hs], axis=1)
        bq = np.concatenate([b3[h, 0, :] for h in hs]) * scale
        bk = np.concatenate([b3[h, 1, :] for h in hs])
        bv_ = np.concatenate([b3[h, 2, :] for h in hs])
        in_maps.append({
            "xT": xT,
            "wqk": np.ascontiguousarray(
                np.concatenate([wq, wk], axis=1)).astype(BF16),
            "wv": np.ascontiguousarray(wv_).astype(BF16),
            "wo": wo_b,
            "bqk": np.concatenate([bq, bk])[None, :].astype(BF16),
            "bv": bv_[None, :].astype(BF16),
            "bo": bo[None, :].astype(BF16),
            "mask": mask,
        })
    return in_maps


def run(x, Wqkv, bqkv, Wo, bo, trace=False):
    from concourse.bass_utils import run_bass_kernel_spmd

    if "nc" not in _CACHE:
        _CACHE["nc"] = _build_program()
    nc = _CACHE["nc"]
    in_maps = _prep_inputs(x, Wqkv, bqkv, Wo, bo)
    res = run_bass_kernel_spmd(nc, in_maps, list(range(NC)), trace=trace)
    # core c returns [512, D]: 4 chunks of 128 rows: (b0 rows 128c..),
    # (b0 rows 1024+128c..), (b1 rows 128c..), (b1 rows 1024+128c..)
    full = np.empty((B, S, D), dtype=np.float32)
    for c in range(NC):
        r = res.results[c]["out"]
        for g in range(4):
            b, half = g // 2, g % 2
            lo = half * 1024 + 128 * c
            full[b, lo:lo + 128, :] = r[g * 128:(g + 1) * 128, :]
    return full, res


def kernel(x, Wqkv, bqkv, Wo, bo):
    out, _ = run(x, Wqkv, bqkv, Wo, bo)
    return out


# revision 29
# speedup vs baseline: 1.4268x; 1.0724x over previous
"""Trainium2 Bass kernel for causal multi-head attention.

Problem: B=2, S=2048, D=1024, H=16 heads (hd=64), fp32 in/out.
  qkv = x @ Wqkv + bqkv ; per-head causal softmax attention ; out = ctx @ Wo + bo

Sharding (8 NeuronCores): tensor-parallel over heads — 2 heads per core.
Each core computes q/k/v projections for its 2 heads (both batches), causal
attention, and its ctx^T slice [128 feat, B*S]. Four AllToAll exchanges
(one per 512-row output chunk) route each core's 128-feature slice of the
other cores' output rows; each core then computes the output projection for
its 512 rows (4 chunks of 128) with the full Wo. Host reassembles.

Schedule: the softmax exp on the Scalar engine paces the scores stream
(0.833ns/elem vs PE's 0.417ns/row-elem), so ACT-independent PE work (the
attn@v of two-windows-ago, qkv projection chunks, output projections) is
woven between score pieces to keep the PE dense and at full p-state clock.

Numerics: bf16 matmul operands, fp32 PSUM accumulation. Softmax uses exp
without max-subtraction (scores ~N(0,1) after the folded 1/sqrt(hd) scale).
The softmax denominator comes free as a ones-column appended to v in the
attn@v matmul. Bias adds are K=1 PE matmuls inside the PSUM accumulation
groups (biases are zero in this problem but kept for fidelity).
"""

import numpy as np
import ml_dtypes

B, S, D, H, NC = 2, 2048, 1024, 16, 8
HD = D // H            # 64
HPC = H // NC          # 2 heads per core
BS = B * S             # 4096
RPB = S // NC          # 256 output rows per core per batch
KC = D // 128          # 8 contraction chunks
SC = BS // 512         # 8 s-chunks for qkv projection
NQT = S // 512         # 4 q-windows (512) per batch
NKT = S // 128         # 16 k-tiles (128) per batch
NCH = 2 * B            # 4 output chunks (b, half)

BF16 = ml_dtypes.bfloat16

_CACHE = {}


def _build_program():
    import concourse.bass as bass
    import concourse.mybir as mybir
    from concourse import bacc
    from concourse.tile import TileContext

    dt = mybir.dt
    f32, bf16 = dt.float32, dt.bfloat16
    ALU = mybir.AluOpType
    ACTF = mybir.ActivationFunctionType

    nc = bacc.Bacc("TRN2", target_bir_lowering=False, debug=False, num_devices=NC)

    xT = nc.dram_tensor("xT", [D, BS], bf16, kind="ExternalInput")
    wqk = nc.dram_tensor("wqk", [D, 256], bf16, kind="ExternalInput")
    wv = nc.dram_tensor("wv", [D, 128], bf16, kind="ExternalInput")
    wo = nc.dram_tensor("wo", [D, D], bf16, kind="ExternalInput")
    bqk = nc.dram_tensor("bqk", [1, 256], bf16, kind="ExternalInput")
    bv = nc.dram_tensor("bv", [1, 128], bf16, kind="ExternalInput")
    bo = nc.dram_tensor("bo", [1, D], bf16, kind="ExternalInput")
    mask = nc.dram_tensor("mask", [128, 896], bf16, kind="ExternalInput")
    out = nc.dram_tensor("out", [NCH * 128, D], f32, kind="ExternalOutput")

    # AllToAll buffers: for chunk g, block d of a2a_in holds my 128 features
    # for destination core d's 128 output rows; a2a_out block s holds core
    # s's 128 features for MY 128 rows of chunk g.
    a2a_in = [nc.dram_tensor(f"a2ain{g}", [NC * 128, 128], bf16)
              for g in range(NCH)]
    a2a_out = [nc.dram_tensor(f"a2aout{g}", [NC * 128, 128], bf16)
               for g in range(NCH)]
    # tiny start-of-program alignment collective: absorbs cross-core launch
    # skew during the DMA-bound startup window instead of mid-stream
    bar_in = nc.dram_tensor("bar_in", [1, 64], bf16)
    bar_out = nc.dram_tensor("bar_out", [NC, 64], bf16)

    with TileContext(nc) as tc:
        with (
            tc.tile_pool(name="const", bufs=1) as cpool,
            tc.tile_pool(name="big", bufs=1) as bigpool,
            tc.tile_pool(name="xstream", bufs=3) as xpool,
            tc.tile_pool(name="exp", bufs=1) as epool,
            tc.tile_pool(name="small", bufs=3) as spool,
            tc.tile_pool(name="agbuf", bufs=2) as agpool,
            tc.tile_pool(name="psA", bufs=2, space="PSUM") as psA,   # 2x [128,1536]
            tc.tile_pool(name="psB", bufs=2, space="PSUM") as psB,   # 2x [128,512]
        ):
            # ---- urgent constants on the sync DMA queue ----
            wqk_sb = cpool.tile([128, KC, 256], bf16, tag="wqk")
            nc.sync.dma_start(wqk_sb[:], wqk.rearrange("(ko p) m -> p ko m", p=128))
            wv_sb = cpool.tile([128, KC, 128], bf16, tag="wv")
            nc.sync.dma_start(wv_sb[:], wv.rearrange("(ko p) m -> p ko m", p=128))
            bqk_sb = cpool.tile([1, 256], bf16, tag="bqk")
            nc.sync.dma_start(bqk_sb[:], bqk[:])
            bv_sb = cpool.tile([1, 128], bf16, tag="bv")
            nc.sync.dma_start(bv_sb[:], bv[:])
            # ---- lazy constants on the gpsimd DMA queue (not needed until
            # masks/proj, keeps the sync queue clear for x streaming) ----
            mask_sb = cpool.tile([128, 896], bf16, tag="mask")
            nc.gpsimd.dma_start(mask_sb[:], mask[:])
            bo_sb = cpool.tile([1, D], bf16, tag="bo")
            nc.gpsimd.dma_start(bo_sb[:], bo[:])
            wo_sb = cpool.tile([128, KC, D], bf16, tag="wo")
            nc.gpsimd.dma_start(wo_sb[:], wo.rearrange("(ko p) m -> p ko m", p=128))

            ones_sb = cpool.tile([1, 512], bf16, tag="ones")
            nc.vector.memset(ones_sb[:], 1.0)
            zrow_sb = cpool.tile([1, 65], bf16, tag="zrow")
            nc.vector.memset(zrow_sb[:], 0.0)

            nc.gpsimd.dma_start(bar_in[:], ones_sb[:, 0:64])
            nc.gpsimd.collective_compute(
                "AllGather", mybir.AluOpType.bypass,
                replica_groups=[list(range(NC))],
                ins=[bar_in[:]], outs=[bar_out[:]],
            )

            # ---- persistent activations ----
            qT_sb = bigpool.tile([128, BS], bf16, tag="qT")   # [2*64 feat, B*S]
            kT_sb = bigpool.tile([128, BS], bf16, tag="kT")
            # v natural layout + ones cols: per 128-row chunk:
            #   [v_h0(0:64) | ones(64) | v_h1(65:129) | ones(129)]
            v_sb = bigpool.tile([128, BS // 128, 130], bf16, tag="v")
            ctxT_sb = bigpool.tile([128, BS], bf16, tag="ctxT")

            nc.vector.memset(v_sb[:, :, 64:65], 1.0)
            nc.vector.memset(v_sb[:, :, 129:130], 1.0)

            xT_r = xT.rearrange("(ko p) s -> p ko s", p=128)

            # x-chunk stream: DMAs all issued upfront on the dedicated sync
            # queue; each waits only for its ring buffer to free (bufs=3)
            xts = {}

            def xt_fetch(sc):
                xts[sc] = xpool.tile([128, KC, 512], bf16, tag="xt",
                                     name="xt")
                nc.sync.dma_start(xts[sc][:],
                                  xT_r[:, :, sc * 512:(sc + 1) * 512])

            # ---- qkv projection chunk, split into PE-filler steps ----
            def qkv_steps(sc):
                xt = xts[sc]
                qs = slice(sc * 512, (sc + 1) * 512)

                def qk_step(col0, dst):
                    ps = psA.tile([128, 1536], f32, tag="psA",
                                  name="ps_qk")[:, :512]
                    for kk in range(KC):
                        nc.tensor.matmul(ps, lhsT=wqk_sb[:, kk, col0:col0 + 128],
                                         rhs=xt[:, kk, :],
                                         start=(kk == 0), stop=False)
                    nc.tensor.matmul(ps, lhsT=bqk_sb[:, col0:col0 + 128],
                                     rhs=ones_sb[:], start=False, stop=True)
                    nc.vector.tensor_copy(dst[:, qs], ps)

                def v_step(s4):
                    sidx = sc * 4 + s4
                    ps_v = psB.tile([128, 512], f32, tag="psB",
                                    name="ps_v")[:, :128]
                    for kk in range(KC):
                        nc.tensor.matmul(
                            ps_v,
                            lhsT=xt[:, kk, s4 * 128:(s4 + 1) * 128],
                            rhs=wv_sb[:, kk, :],
                            start=(kk == 0), stop=False)
                    nc.tensor.matmul(ps_v, lhsT=ones_sb[:, 0:128], rhs=bv_sb[:],
                                     start=False, stop=True)
                    nc.vector.tensor_copy(v_sb[:, sidx, 0:64], ps_v[:, 0:64])
                    nc.vector.tensor_copy(v_sb[:, sidx, 65:129], ps_v[:, 64:128])

                return [lambda: qk_step(0, qT_sb), lambda: qk_step(128, kT_sb),
                        lambda: (v_step(0), v_step(1)),
                        lambda: (v_step(2), v_step(3))]

            # ---- output projection for chunk g, split into 2 steps ----
            def proj_steps(g):
                ctxag_sb = agpool.tile([128, NC, 128], bf16, tag="ctxag",
                                       name="ctxag_sb")
                nc.gpsimd.dma_start(
                    ctxag_sb[:], a2a_out[g].rearrange("(k p) s -> p k s", p=128))
                ot = agpool.tile([128, D], f32, tag="ot")

                def ncol_step(ncol, last):
                    ps_o = psB.tile([128, 512], f32, tag="psB", name="ps_o")
                    for k in range(NC):
                        nc.tensor.matmul(
                            ps_o,
                            lhsT=ctxag_sb[:, k, :],
                            rhs=wo_sb[:, k, ncol * 512:(ncol + 1) * 512],
                            start=(k == 0), stop=False)
                    nc.tensor.matmul(
                        ps_o, lhsT=ones_sb[:, 0:128],
                        rhs=bo_sb[:, ncol * 512:(ncol + 1) * 512],
                        start=False, stop=True)
                    nc.vector.tensor_copy(
                        ot[:, ncol * 512:(ncol + 1) * 512], ps_o)
                    if last:
                        nc.gpsimd.dma_start(out[g * 128:(g + 1) * 128, :],
                                            ot[:])

                return [lambda: ncol_step(0, False), lambda: ncol_step(1, True)]

            # band piece layout: 4 staggered sub-pieces [o*128:512] of the
            # diagonal band, packed at offsets BOFF with widths BWID.
            # Offsets chosen so no matmul output crosses a 512-col PSUM bank.
            BOFF = [0, 512, 1024, 896]
            BWID = [512, 384, 256, 128]
            BTOT = 1280

            # ---- attn@v for one (batch, head, window), as filler steps ----
            def ctx_steps(b, hl, j, exp_j):
                hp = slice(64 * hl, 64 * hl + 64)
                nr = 4 * j
                state = {}

                def band_mm(ps_c, o, start, stop, skip=True):
                    nc.tensor.matmul(
                        ps_c[:65, o * 128:512],
                        lhsT=v_sb[:, b * NKT + nr + o, 65 * hl: 65 * hl + 65],
                        rhs=exp_j[:, nr * 512 + BOFF[o]:
                                  nr * 512 + BOFF[o] + BWID[o]],
                        start=start, stop=stop, skip_group_check=skip)

                def mm_run(lo, hi):
                    if lo == 0:
                        state["ps_c"] = psB.tile([128, 512], f32, tag="psB",
                                                 name="ps_c")
                    ps_c = state["ps_c"]
                    for i in range(lo, hi):
                        if nr > 0:
                            # order: rect tiles, bands 1..3, band 0 last
                            if i < nr:
                                nc.tensor.matmul(
                                    ps_c[:65, :],
                                    lhsT=v_sb[:, b * NKT + i,
                                              65 * hl: 65 * hl + 65],
                                    rhs=exp_j[:, i * 512:(i + 1) * 512],
                                    start=(i == 0), stop=False,
                                    skip_group_check=(i > 0))
                            elif i < nr + 3:
                                band_mm(ps_c, i - nr + 1, False, False)
                            else:
                                band_mm(ps_c, 0, False, True, skip=False)
                        else:
                            # j == 0: band 0 first, then 1..3, then closer
                            if i == 0:
                                band_mm(ps_c, 0, True, False, skip=False)
                            elif i < 4:
                                band_mm(ps_c, i, False, False)
                            else:
                                nc.tensor.matmul(
                                    ps_c[:65, :], lhsT=zrow_sb[:],
                                    rhs=ones_sb[:], start=False, stop=True)

                def finalize():
                    ps_c = state["ps_c"]
                    den = spool.tile([1, 512], f32, tag="den")
                    nc.vector.tensor_copy(den[:], ps_c[64:65, :])
                    recip = spool.tile([1, 512], f32, tag="recip")
                    nc.vector.reciprocal_approx_fast(out=recip[:], in_=den[:])
                    bcast = spool.tile([128, 512], f32, tag="bcast")
                    nc.gpsimd.partition_broadcast(bcast[:], recip[:])
                    cs = slice(b * S + j * 512, b * S + (j + 1) * 512)
                    nc.vector.tensor_tensor(
                        ctxT_sb[hp, cs], ps_c[0:64, :], bcast[0:64, :],
                        ALU.mult)
                    if hl == 1 and j in (1, 3):
                        emit_a2a(2 * b + (j == 3))

                nmm = (nr + 4) if nr > 0 else 5
                steps = []
                lo = 0
                while lo < nmm:
                    hi = min(lo + 5, nmm)
                    steps.append(lambda lo=lo, hi=hi: mm_run(lo, hi))
                    lo = hi
                steps.append(finalize)
                return steps

            def emit_a2a(g):
                lo = g * 1024
                nc.gpsimd.dma_start(
                    a2a_in[g].rearrange("(d p) c -> p d c", p=128),
                    ctxT_sb[:, lo:lo + 1024].rearrange("p (d c) -> p d c", d=NC))
                nc.gpsimd.collective_compute(
                    "AllToAll",
                    mybir.AluOpType.bypass,
                    replica_groups=[list(range(NC))],
                    ins=[a2a_in[g][:]],
                    outs=[a2a_out[g][:]],
                )

            # ---- schedule ----
            pending = []
            # proj chunks at window END (never ahead of scores in the PE
            # queue - a late AllToAll then cannot block the window's work)
            proj_end = {(1, 2): 0, (1, 3): 1}
            qkv_at = {(0, 0): [1], (0, 1): [2], (0, 2): [3, 4],
                      (0, 3): [5], (1, 0): [6], (1, 1): [7]}
            # fire chunk-0 A2A one window early; drain the j2 ctx units
            # inside window (1,3) to shorten the tail
            pop_to = {(0, 2): 2, (1, 3): 2}

            scope1 = nc.named_scope("qkv"); scope1.__enter__()
            xt_fetch(0)
            for st in qkv_steps(0):
                st()
            for sc_i in range(1, SC):
                xt_fetch(sc_i)
            scope1.__exit__(None, None, None)

            for b in range(B):
                for j in range(NQT):
                    scope = nc.named_scope(f"w{b}{j}"); scope.__enter__()
                    nr = 4 * j
                    exp_js = [epool.tile([128, nr * 512 + BTOT], bf16,
                                         tag=f"expj{j}h{hl}", name="exp_j")
                              for hl in range(HPC)]
                    qwins = [qT_sb[slice(64 * hl, 64 * hl + 64),
                                   b * S + j * 512: b * S + (j + 1) * 512]
                             for hl in range(HPC)]

                    # scores pieces (ACT-paced): rect pieces then band pieces
                    def rect_piece(hl, tt, npc):
                        hp = slice(64 * hl, 64 * hl + 64)
                        ps = psA.tile([128, 1536], f32, tag="psA",
                                      name="ps_sc")[:, :npc * 512]
                        for i in range(npc):
                            kt = b * S + (tt + i) * 128
                            nc.tensor.matmul(
                                ps[:, i * 512:(i + 1) * 512],
                                lhsT=kT_sb[hp, kt:kt + 128],
                                rhs=qwins[hl],
                                start=True, stop=True)
                        nc.scalar.activation(
                            exp_js[hl][:, tt * 512:(tt + npc) * 512],
                            ps, ACTF.Exp)

                    def band_piece(hl):
                        hp = slice(64 * hl, 64 * hl + 64)
                        ps = psA.tile([128, 1536], f32, tag="psA",
                                      name="ps_band")[:, :BTOT]
                        for o in range(4):
                            kt = b * S + (nr + o) * 128
                            nc.tensor.matmul(
                                ps[:, BOFF[o]:BOFF[o] + BWID[o]],
                                lhsT=kT_sb[hp, kt:kt + 128],
                                rhs=qwins[hl][:, o * 128:512],
                                start=True, stop=True)
                        nc.scalar.activation(
                            exp_js[hl][:, nr * 512:nr * 512 + BTOT],
                            ps, ACTF.Exp)
                        for o in range(4):
                            blk = slice(nr * 512 + BOFF[o],
                                        nr * 512 + BOFF[o] + 128)
                            nc.vector.tensor_tensor(exp_js[hl][:, blk],
                                                    exp_js[hl][:, blk],
                                                    mask_sb[:, 384:512],
                                                    ALU.mult)

                    pieces = []
                    tt = 0
                    while tt < nr:
                        npc = min(3, nr - tt)
                        for hl in range(HPC):
                            pieces.append(
                                lambda hl=hl, tt=tt, npc=npc:
                                rect_piece(hl, tt, npc))
                        tt += npc
                    for hl in range(HPC):
                        pieces.append(lambda hl=hl: band_piece(hl))

                    # ACT-independent PE fillers
                    fillers = []
                    for sc_i in qkv_at.get((b, j), []):
                        fillers += qkv_steps(sc_i)
                    for hl in range(HPC):
                        pending.append((b, hl, j, exp_js[hl]))
                    thresh = pop_to.get((b, j), 4)
                    while len(pending) > thresh:
                        fillers += ctx_steps(*pending.pop(0))

                    # weave: scores pieces paced by ACT, fillers between
                    fi = 0
                    for pi, piece in enumerate(pieces):
                        piece()
                        rem_p = len(pieces) - pi - 1
                        want = ((len(fillers) - fi) + rem_p) // (rem_p + 1)
                        for _ in range(want):
                            fillers[fi](); fi += 1
                    while fi < len(fillers):
                        fillers[fi](); fi += 1
                    if (b, j) in proj_end:
                        for st in proj_steps(proj_end[(b, j)]):
                            st()
                    scope.__exit__(None, None, None)

            scope3 = nc.named_scope("tail"); scope3.__enter__()
            # interleave the remaining ctx units pairwise so the PE keeps
            # matmuls in flight while each unit's normalize chain drains
            while pending:
                pair = [ctx_steps(*pending.pop(0))]
                if pending:
                    pair.append(ctx_steps(*pending.pop(0)))
                n = max(len(p) for p in pair)
                for i in range(n):
                    for p in pair:
                        if i < len(p):
                            p[i]()
            for st in proj_steps(2) + proj_steps(3):
                st()
            scope3.__exit__(None, None, None)

    nc.compile()
    return nc


def _prep_inputs(x, Wqkv, bqkv, Wo, bo):
    x = np.asarray(x, dtype=np.float32)
    Wqkv = np.asarray(Wqkv, dtype=np.float32)
    bqkv = np.asarray(bqkv, dtype=np.float32)
    Wo = np.asarray(Wo, dtype=np.float32)
    bo = np.asarray(bo, dtype=np.float32)

    xT = np.ascontiguousarray(x.reshape(BS, D).T).astype(BF16)
    wo_b = Wo.astype(BF16)

    kp = np.arange(128)[:, None]
    u = np.arange(896)[None, :]
    mask = (u >= 384 + kp).astype(BF16)

    scale = np.float32(1.0 / np.sqrt(HD))

    # Wqkv columns per head h: q = 192h..+64, k = +64, v = +128
    W3 = Wqkv.reshape(D, H, 3, HD)
    b3 = bqkv.reshape(H, 3, HD)

    in_maps = []
    for c in range(NC):
        hs = [HPC * c + i for i in range(HPC)]
        wq = np.concatenate([W3[:, h, 0, :] for h in hs], axis=1) * scale
        wk = np.concatenate([W3[:, h, 1, :] for h in hs], axis=1)
        wv_ = np.concatenate([W3[:, h, 2, :] for h in hs], axis=1)
        bq = np.concatenate([b3[h, 0, :] for h in hs]) * scale
        bk = np.concatenate([b3[h, 1, :] for h in hs])
        bv_ = np.concatenate([b3[h, 2, :] for h in hs])
        in_maps.append({
            "xT": xT,
            "wqk": np.ascontiguousarray(
                np.concatenate([wq, wk], axis=1)).astype(BF16),
            "wv": np.ascontiguousarray(wv_).astype(BF16),
            "wo": wo_b,
            "bqk": np.concatenate([bq, bk])[None, :].astype(BF16),
            "bv": bv_[None, :].astype(BF16),
            "bo": bo[None, :].astype(BF16),
            "mask": mask,
        })
    return in_maps


def run(x, Wqkv, bqkv, Wo, bo, trace=False):
    from concourse.bass_utils import run_bass_kernel_spmd

    if "nc" not in _CACHE:
        _CACHE["nc"] = _build_program()
    nc = _CACHE["nc"]
    in_maps = _prep_inputs(x, Wqkv, bqkv, Wo, bo)
    res = run_bass_kernel_spmd(nc, in_maps, list(range(NC)), trace=trace)
    # core c returns [512, D]: 4 chunks of 128 rows: (b0 rows 128c..),
    # (b0 rows 1024+128c..), (b1 rows 128c..), (b1 rows 1024+128c..)
    full = np.empty((B, S, D), dtype=np.float32)
    for c in range(NC):
        r = res.results[c]["out"]
        for g in range(4):
            b, half = g // 2, g % 2
            lo = half * 1024 + 128 * c
            full[b, lo:lo + 128, :] = r[g * 128:(g + 1) * 128, :]
    return full, res


def kernel(x, Wqkv, bqkv, Wo, bo):
    out, _ = run(x, Wqkv, bqkv, Wo, bo)
    return out
